# revision 15
# baseline (speedup 1.0000x reference)
"""Bass/Trainium2 kernel for windowed multi-head attention (8 NeuronCores).

Math per window b (64 tokens), matching torch-style nn.MultiHeadAttention:
  qh = (q @ Wq.T + bq) * scale; kh = k @ Wk.T + bk; vh = v @ Wv.T + bv
  S  = qh @ kh.T  (per head);  S[masked k] = -1e4;  P = softmax(S)
  out = concat_h(P @ vh) @ Wp.T + bp

Device dataflow (per core, 256 windows = 16384 tokens, all matmuls bf16
with f32 PSUM accumulation):
  - q,k,v are pre-transposed on the host to feature-major [128, 4, TT]
    T-tile blocks so input DMAs are single fully-contiguous 512KB reads.
  - Projections on PE: q -> feature-major Y.T chunks; k -> feature-major
    zero-padded per-head-slot layout; v -> token-major.
  - Per window-pair (128 tokens): S for all 8 heads via 8 K=128 matmuls
    (the zero-padded k layout keeps the two heads of a chunk separate),
    additive mask applied as one rank-2 matmul (sel01 x maskrep),
    exp on ACT, per-head row-sum + reciprocal + normalize on DVE writing
    a zero-padded P layout, P.T via 8 DMA-xbar transposes (split across
    both HWDGE queues), X.T via 8 K=128 matmuls against block-diagonal
    P.T, then the output projection directly from X.T chunks.

All matmul operands sit at partition base 0 with K in {2, 128}: mixing
row-group offsets across back-to-back K<128 matmuls crashes the PE.

Biases are zero in this problem; nonzero bv/bp fold into a host-side
output add, nonzero bq/bk fall back to a host computation.
"""

import sys

for _p in ("/opt/trn_rl_repo",):
    if _p not in sys.path:
        sys.path.append(_p)

import numpy as np
import ml_dtypes

import concourse.bacc as bacc
import concourse.bass as bass
import concourse.mybir as mybir
import concourse.tile as tile
from concourse.bass import ds, ts
from concourse.bass_utils import run_bass_kernel_spmd

BF16 = mybir.dt.bfloat16
F32 = mybir.dt.float32
NP_BF16 = ml_dtypes.bfloat16

NCORES = 8
DIM = 512
HEADS = 8
HD = 64
N = 64  # window length
B_TOTAL = 2048
WIN_PER_CORE = B_TOTAL // NCORES  # 256
TOK_PER_CORE = WIN_PER_CORE * N  # 16384
SCALE = HD ** -0.5
MASK_NEG = -10000.0
TT = 512  # token tile (8 windows, 4 pairs)


def build_program(win_per_core=WIN_PER_CORE):
    """Build the per-core Tile program. All 8 cores run it SPMD."""
    tok = win_per_core * N
    n_tt = tok // TT
    PAIRS = TT // 128  # 4

    nc = bacc.Bacc("TRN2", debug=False)

    qT_d = nc.dram_tensor("qT", [n_tt, 128, 4, TT], BF16, kind="ExternalInput")
    kT_d = nc.dram_tensor("kT", [n_tt, 128, 4, TT], BF16, kind="ExternalInput")
    vT_d = nc.dram_tensor("vT", [n_tt, 128, 4, TT], BF16, kind="ExternalInput")
    maskrep = nc.dram_tensor(
        "maskrep", [win_per_core, DIM], BF16, kind="ExternalInput"
    )
    wq = nc.dram_tensor("wq", [DIM, DIM], BF16, kind="ExternalInput")
    wk = nc.dram_tensor("wk", [DIM, DIM], BF16, kind="ExternalInput")
    wv = nc.dram_tensor("wv", [DIM, DIM], BF16, kind="ExternalInput")
    wp = nc.dram_tensor("wp", [DIM, DIM], BF16, kind="ExternalInput")
    sel01 = nc.dram_tensor("sel01", [2, 128], BF16, kind="ExternalInput")
    out = nc.dram_tensor("out", [tok, DIM], F32, kind="ExternalOutput")

    qa, ka, va, ma = qT_d.ap(), kT_d.ap(), vT_d.ap(), maskrep.ap()
    oa = out.ap()

    with tile.TileContext(nc) as tc:
        with (
            tc.tile_pool(name="consts", bufs=1) as consts,
            tc.tile_pool(name="pin", bufs=2) as pin,
            tc.tile_pool(name="py", bufs=2) as py,
            tc.tile_pool(name="pmask", bufs=2) as pmask,
            tc.tile_pool(name="pp", bufs=3) as pp,
            tc.tile_pool(name="pst", bufs=4) as pst,
            tc.tile_pool(name="pout", bufs=3) as pout,
            tc.tile_pool(name="ps_pj", bufs=2, space="PSUM") as ps_pj,
            tc.tile_pool(name="ps_s", bufs=2, space="PSUM") as ps_s,
            tc.tile_pool(name="ps_x", bufs=2, space="PSUM") as ps_x,
            tc.tile_pool(name="ps_o", bufs=2, space="PSUM") as ps_o,
        ):
            # Weights: [fi_part 128, fi_chunk 4, fo 512]
            w_tiles = {}
            for name, t in (("wq", wq), ("wk", wk), ("wv", wv), ("wp", wp)):
                wt = consts.tile([128, 4, DIM], BF16, tag=f"w_{name}")
                nc.sync.dma_start(
                    out=wt, in_=t.ap().rearrange("(c p) f -> p c f", p=128)
                )
                w_tiles[name] = wt
            sel_t = consts.tile([2, 128], BF16, tag="sel01")
            nc.sync.dma_start(out=sel_t, in_=sel01.ap())

            for tt_i in range(n_tt):
                t0 = tt_i * TT
                # ---- transposed input loads (host pre-transposed) ----
                qT = pin.tile([128, 4, TT], BF16, tag="qT")
                kT = pin.tile([128, 4, TT], BF16, tag="kT")
                vT = pin.tile([128, 4, TT], BF16, tag="vT")
                nc.sync.dma_start(out=qT, in_=qa[tt_i])
                nc.sync.dma_start(out=kT, in_=ka[tt_i])
                nc.sync.dma_start(out=vT, in_=va[tt_i])
                # [w-in-pair 2, pair, 512] so rhs slices sit at partition 0
                mt = pmask.tile([2, PAIRS, DIM], BF16, tag="mt")
                nc.sync.dma_start(
                    out=mt,
                    in_=ma[tt_i * 2 * PAIRS : (tt_i + 1) * 2 * PAIRS, :].rearrange(
                        "(p w) f -> w p f", w=2
                    ),
                )

                # ---- q projection -> feature-major bf16 [fo 128, c, tok] ----
                yqT = py.tile([128, 4, TT], BF16, tag="yqT")
                for co in range(4):
                    ps = ps_pj.tile([128, TT], F32, tag="pj")
                    for ci in range(4):
                        nc.tensor.matmul(
                            ps,
                            lhsT=w_tiles["wq"][:, ci, ts(co, 128)],
                            rhs=qT[:, ci, :],
                            start=(ci == 0),
                            stop=(ci == 3),
                        )
                    nc.scalar.copy(out=yqT[:, co, :], in_=ps)

                # ---- k projection -> zero-padded per-head-slot layout ----
                # ykTz[0:64, c, 0, :] = head 2c rows, ykTz[64:128, c, 1, :] =
                # head 2c+1 rows, other slots zero -> K=128 S matmuls keep
                # the two heads of a chunk separate.
                ykTz = py.tile([128, 4, 2, TT], BF16, tag="ykTz")
                nc.gpsimd.memset(ykTz[0:64, :, 1, :], 0.0)
                nc.gpsimd.memset(ykTz[64:128, :, 0, :], 0.0)
                for co in range(4):
                    ps = ps_pj.tile([128, TT], F32, tag="pj")
                    for ci in range(4):
                        nc.tensor.matmul(
                            ps,
                            lhsT=w_tiles["wk"][:, ci, ts(co, 128)],
                            rhs=kT[:, ci, :],
                            start=(ci == 0),
                            stop=(ci == 3),
                        )
                    nc.scalar.copy(out=ykTz[0:64, co, 0, :], in_=ps[0:64, :])
                    nc.scalar.copy(out=ykTz[64:128, co, 1, :], in_=ps[64:128, :])

                # ---- v projection -> token-major bf16 [tok%128, pair, fo] ----
                yv = py.tile([128, PAIRS, DIM], BF16, tag="yv")
                for ct in range(PAIRS):
                    ps = ps_pj.tile([128, DIM], F32, tag="pj")
                    for ci in range(4):
                        nc.tensor.matmul(
                            ps,
                            lhsT=vT[:, ci, ts(ct, 128)],
                            rhs=w_tiles["wv"][:, ci, :],
                            start=(ci == 0),
                            stop=(ci == 3),
                        )
                    nc.scalar.copy(out=yv[:, ct, :], in_=ps)

                # ---- attention + output projection per 128-token pair ----
                for pr in range(PAIRS):
                    base = pr * 128
                    # S: [pair-tq 128, (head, tk) 512]
                    sp = ps_s.tile([128, HEADS, N], F32, tag="sp")
                    nc.tensor.matmul(
                        sp,
                        lhsT=sel_t,
                        rhs=mt[:, pr, :],
                        start=True,
                        stop=False,
                        skip_group_check=True,
                    )
                    for ch in range(4):
                        for w in range(2):
                            nc.tensor.matmul(
                                sp[ds(64 * w, 64), ds(2 * ch, 2), :],
                                lhsT=yqT[:, ch, ds(base + 64 * w, 64)],
                                rhs=ykTz[:, ch, :, ds(base + 64 * w, 64)],
                                start=False,
                                stop=True,
                                skip_group_check=True,
                            )
                    # softmax (no max-subtraction needed: |S| < 2 or -1e4)
                    pexp = pp.tile([128, HEADS, N], F32, tag="pexp")
                    nc.scalar.activation(
                        out=pexp, in_=sp, func=mybir.ActivationFunctionType.Exp
                    )
                    dsum = pst.tile([128, HEADS], F32, tag="dsum")
                    nc.vector.tensor_reduce(
                        out=dsum,
                        in_=pexp,
                        axis=mybir.AxisListType.X,
                        op=mybir.AluOpType.add,
                    )
                    rec = pst.tile([128, HEADS], F32, tag="rec")
                    nc.vector.reciprocal(out=rec, in_=dsum)
                    # normalized P in zero-padded window-slot layout:
                    # pnz[(w,tq), h, wc, tk] = P_wh[tq, tk] iff wc == w
                    pnz = pp.tile([128, HEADS, 2, N], BF16, tag="pnz")
                    nc.gpsimd.memset(pnz[0:64, :, 1, :], 0.0)
                    nc.gpsimd.memset(pnz[64:128, :, 0, :], 0.0)
                    for w in range(2):
                        rec_s = rec[ds(64 * w, 64), :]
                        rec_b = bass.AP(
                            tensor=rec_s.tensor,
                            offset=rec_s.offset,
                            ap=list(rec_s.ap) + [[0, N]],
                        )
                        nc.vector.tensor_tensor(
                            out=pnz[ds(64 * w, 64), :, w, :],
                            in0=pexp[ds(64 * w, 64), :, :],
                            in1=rec_b,
                            op=mybir.AluOpType.mult,
                        )
                    # P.T per head via DMA xbar: [128 (wc, tk), 128 (w, tq)],
                    # block-diagonal. Split across both HWDGE queues.
                    pTz = pp.tile([128, HEADS, 128], BF16, tag="pTz")
                    pnz2 = pnz.rearrange("p h w n -> p (h w n)")
                    for h in range(HEADS):
                        eng = nc.sync
                        eng.dma_start(
                            out=pTz[:, h, :],
                            in_=pnz2[:, ts(h, 128)],
                            transpose=True,
                        )
                    # X.T: [feat-in-chunk 128, chunk 4, (w, tq) 128]
                    xt = ps_x.tile([128, 4, 128], F32, tag="xt")
                    for h in range(HEADS):
                        ch, hh = h // 2, (h % 2) * 64
                        nc.tensor.matmul(
                            xt[ds(hh, 64), ch, :],
                            lhsT=yv[:, pr, ts(h, 64)],
                            rhs=pTz[:, h, :],
                            start=True,
                            stop=True,
                        )
                    xts = pp.tile([128, 4, 128], BF16, tag="xts")
                    nc.scalar.copy(out=xts, in_=xt)
                    # output projection: [tok 128, fo 512]
                    po = ps_o.tile([128, DIM], F32, tag="po")
                    for c in range(4):
                        nc.tensor.matmul(
                            po,
                            lhsT=xts[:, c, :],
                            rhs=w_tiles["wp"][:, c, :],
                            start=(c == 0),
                            stop=(c == 3),
                        )
                    osb = pout.tile([128, DIM], F32, tag="osb")
                    nc.scalar.copy(out=osb, in_=po)
                    nc.sync.dma_start(
                        out=oa[t0 + base : t0 + base + 128, :], in_=osb
                    )

    nc.compile()
    return nc


_PROGRAM_CACHE = {}


def _get_program(win_per_core):
    if win_per_core not in _PROGRAM_CACHE:
        _PROGRAM_CACHE[win_per_core] = build_program(win_per_core)
    return _PROGRAM_CACHE[win_per_core]


def _feature_major_tiles(x_flat_bf16):
    """[tok, 512] bf16 -> [n_tt, 128, 4, TT] so each T-tile block is one
    fully-contiguous transposed DMA read."""
    tok = x_flat_bf16.shape[0]
    n_tt = tok // TT
    xt = x_flat_bf16.reshape(n_tt, TT, 4, 128).transpose(0, 3, 2, 1)
    return np.ascontiguousarray(xt)


def make_in_maps(q, k, v, mask, Wq, Wk, Wv, Wp, ncores=NCORES):
    """Host-side shard + layout prep. Returns list of per-core input dicts."""
    B, n, C = q.shape
    win_pc = B // ncores
    qf = np.ascontiguousarray(q.reshape(B * n, C))
    kf = np.ascontiguousarray(k.reshape(B * n, C))
    vf = np.ascontiguousarray(v.reshape(B * n, C))

    # additive mask, repeated per head along the free dim
    madd = np.where(mask == 0, np.float32(MASK_NEG), np.float32(0.0))
    maskrep = np.tile(madd, (1, HEADS)).astype(NP_BF16)  # [B, 512]

    wq_t = np.ascontiguousarray((Wq.T * SCALE).astype(NP_BF16))
    wk_t = np.ascontiguousarray(Wk.T.astype(NP_BF16))
    wv_t = np.ascontiguousarray(Wv.T.astype(NP_BF16))
    wp_t = np.ascontiguousarray(Wp.T.astype(NP_BF16))
    sel = np.zeros((2, 128), NP_BF16)
    sel[0, 0:64] = 1
    sel[1, 64:128] = 1

    tok_pc = win_pc * n
    in_maps = []
    for c in range(ncores):
        sl = slice(c * tok_pc, (c + 1) * tok_pc)
        in_maps.append(
            {
                "qT": _feature_major_tiles(qf[sl].astype(NP_BF16)),
                "kT": _feature_major_tiles(kf[sl].astype(NP_BF16)),
                "vT": _feature_major_tiles(vf[sl].astype(NP_BF16)),
                "maskrep": maskrep[c * win_pc : (c + 1) * win_pc],
                "wq": wq_t,
                "wk": wk_t,
                "wv": wv_t,
                "wp": wp_t,
                "sel01": sel,
            }
        )
    return in_maps


def _reference_numpy(q, k, v, mask, Wq, bq, Wk, bk, Wv, bv, Wp, bp):
    """Full-precision host fallback (only used for nonzero bq/bk)."""
    B, n, C = q.shape
    qh = (q.reshape(-1, C) @ Wq.T + bq).reshape(B, n, HEADS, HD).transpose(0, 2, 1, 3)
    kh = (k.reshape(-1, C) @ Wk.T + bk).reshape(B, n, HEADS, HD).transpose(0, 2, 1, 3)
    vh = (v.reshape(-1, C) @ Wv.T + bv).reshape(B, n, HEADS, HD).transpose(0, 2, 1, 3)
    s = np.einsum("bhqd,bhkd->bhqk", qh * SCALE, kh)
    s = np.where((mask[:, None, None, :] == 0), np.float32(MASK_NEG), s)
    s = s - s.max(-1, keepdims=True)
    e = np.exp(s)
    p = e / e.sum(-1, keepdims=True)
    x = np.einsum("bhqk,bhkd->bhqd", p, vh)
    x = x.transpose(0, 2, 1, 3).reshape(B, n, C)
    return (x @ Wp.T + bp).astype(np.float32)


def kernel(q, k, v, mask, Wq, bq, Wk, bk, Wv, bv, Wp, bp, trace=False):
    q = np.asarray(q, np.float32)
    k = np.asarray(k, np.float32)
    v = np.asarray(v, np.float32)
    mask = np.asarray(mask)
    Wq, Wk, Wv, Wp = (np.asarray(w, np.float32) for w in (Wq, Wk, Wv, Wp))
    bq, bk, bv, bp = (np.asarray(b, np.float32) for b in (bq, bk, bv, bp))

    if np.any(bq) or np.any(bk):
        return _reference_numpy(q, k, v, mask, Wq, bq, Wk, bk, Wv, bv, Wp, bp)

    B, n, C = q.shape
    win_pc = B // NCORES
    nc = _get_program(win_pc)
    in_maps = make_in_maps(q, k, v, mask, Wq, Wk, Wv, Wp)
    res = run_bass_kernel_spmd(
        nc, in_maps, core_ids=list(range(NCORES)), trace=trace
    )
    outs = np.concatenate([r["out"] for r in res.results], axis=0)
    outs = outs.reshape(B, n, C)
    # bv flows through attention linearly (softmax rows sum to 1); with bp it
    # folds into a single output bias.
    bout = bp + bv @ Wp.T
    if np.any(bout):
        outs = outs + bout.astype(np.float32)
    if trace:
        kernel._last_result = res
    return outs


# revision 17
# speedup vs baseline: 2.6622x; 2.6622x over previous
"""Bass/Trainium2 kernel for windowed multi-head attention (8 NeuronCores).

Math per window b (64 tokens), matching torch-style nn.MultiHeadAttention:
  qh = (q @ Wq.T + bq) * scale; kh = k @ Wk.T + bk; vh = v @ Wv.T + bv
  S  = qh @ kh.T  (per head);  S[masked k] = -1e4;  P = softmax(S)
  out = concat_h(P @ vh) @ Wp.T + bp

Device dataflow (per core, 256 windows = 16384 tokens, all matmuls bf16
with f32 PSUM accumulation):
  - q,k,v are pre-transposed on the host to feature-major [128, 4, TT]
    T-tile blocks so input DMAs are single fully-contiguous 512KB reads.
  - Projections on PE: q -> feature-major Y.T chunks; k -> feature-major
    zero-padded per-head-slot layout; v -> token-major.
  - Per window-pair (128 tokens): S for all 8 heads via 8 K=128 matmuls
    (the zero-padded k layout keeps the two heads of a chunk separate),
    additive mask applied as one rank-2 matmul (sel01 x maskrep),
    exp on ACT, per-head row-sum + reciprocal + normalize on DVE writing
    a zero-padded P layout, P.T via 8 DMA-xbar transposes (split across
    both HWDGE queues), X.T via 8 K=128 matmuls against block-diagonal
    P.T, then the output projection directly from X.T chunks.

All matmul operands sit at partition base 0 with K in {2, 128}: mixing
row-group offsets across back-to-back K<128 matmuls crashes the PE.

Biases are zero in this problem; nonzero bv/bp fold into a host-side
output add, nonzero bq/bk fall back to a host computation.
"""

import sys

for _p in ("/opt/trn_rl_repo",):
    if _p not in sys.path:
        sys.path.append(_p)

import numpy as np
import ml_dtypes

import concourse.bacc as bacc
import concourse.bass as bass
import concourse.mybir as mybir
import concourse.tile as tile
from concourse.bass import ds, ts
from concourse.bass_utils import run_bass_kernel_spmd

BF16 = mybir.dt.bfloat16
F32 = mybir.dt.float32
NP_BF16 = ml_dtypes.bfloat16

NCORES = 8
DIM = 512
HEADS = 8
HD = 64
N = 64  # window length
B_TOTAL = 2048
WIN_PER_CORE = B_TOTAL // NCORES  # 256
TOK_PER_CORE = WIN_PER_CORE * N  # 16384
SCALE = HD ** -0.5
MASK_NEG = -10000.0
TT = 512  # token tile (8 windows, 4 pairs)


def build_program(win_per_core=WIN_PER_CORE):
    """Build the per-core Tile program. All 8 cores run it SPMD."""
    tok = win_per_core * N
    n_tt = tok // TT
    PAIRS = TT // 128  # 4

    nc = bacc.Bacc("TRN2", debug=False)

    qT_d = nc.dram_tensor("qT", [n_tt, 128, 4, TT], BF16, kind="ExternalInput")
    kT_d = nc.dram_tensor("kT", [n_tt, 128, 4, TT], BF16, kind="ExternalInput")
    vT_d = nc.dram_tensor("vT", [n_tt, 128, 4, TT], BF16, kind="ExternalInput")
    maskrep = nc.dram_tensor(
        "maskrep", [win_per_core, DIM], BF16, kind="ExternalInput"
    )
    wq = nc.dram_tensor("wq", [DIM, DIM], BF16, kind="ExternalInput")
    wk = nc.dram_tensor("wk", [DIM, DIM], BF16, kind="ExternalInput")
    wv = nc.dram_tensor("wv", [DIM, DIM], BF16, kind="ExternalInput")
    wp = nc.dram_tensor("wp", [DIM, DIM], BF16, kind="ExternalInput")
    sel01 = nc.dram_tensor("sel01", [2, 128], BF16, kind="ExternalInput")
    ident = nc.dram_tensor("ident", [128, 128], BF16, kind="ExternalInput")
    out = nc.dram_tensor("out", [tok, DIM], F32, kind="ExternalOutput")

    qa, ka, va, ma = qT_d.ap(), kT_d.ap(), vT_d.ap(), maskrep.ap()
    oa = out.ap()

    with tile.TileContext(nc) as tc:
        with (
            tc.tile_pool(name="consts", bufs=1) as consts,
            tc.tile_pool(name="pin", bufs=2) as pin,
            tc.tile_pool(name="py", bufs=2) as py,
            tc.tile_pool(name="pmask", bufs=2) as pmask,
            tc.tile_pool(name="pp", bufs=3) as pp,
            tc.tile_pool(name="pst", bufs=4) as pst,
            tc.tile_pool(name="pout", bufs=3) as pout,
            tc.tile_pool(name="ps_pj", bufs=2, space="PSUM") as ps_pj,
            tc.tile_pool(name="ps_s", bufs=2, space="PSUM") as ps_s,
            tc.tile_pool(name="ps_t", bufs=2, space="PSUM") as ps_t,
            tc.tile_pool(name="ps_x", bufs=1, space="PSUM") as ps_x,
            tc.tile_pool(name="ps_o", bufs=1, space="PSUM") as ps_o,
        ):
            # Weights: [fi_part 128, fi_chunk 4, fo 512]
            w_tiles = {}
            for name, t in (("wq", wq), ("wk", wk), ("wv", wv), ("wp", wp)):
                wt = consts.tile([128, 4, DIM], BF16, tag=f"w_{name}")
                nc.sync.dma_start(
                    out=wt, in_=t.ap().rearrange("(c p) f -> p c f", p=128)
                )
                w_tiles[name] = wt
            sel_t = consts.tile([2, 128], BF16, tag="sel01")
            nc.sync.dma_start(out=sel_t, in_=sel01.ap())
            id_t = consts.tile([128, 128], BF16, tag="ident")
            nc.sync.dma_start(out=id_t, in_=ident.ap())

            for tt_i in range(n_tt):
                t0 = tt_i * TT
                # ---- transposed input loads (host pre-transposed) ----
                qT = pin.tile([128, 4, TT], BF16, tag="qT")
                kT = pin.tile([128, 4, TT], BF16, tag="kT")
                vT = pin.tile([128, 4, TT], BF16, tag="vT")
                nc.sync.dma_start(out=qT, in_=qa[tt_i])
                nc.sync.dma_start(out=kT, in_=ka[tt_i])
                nc.sync.dma_start(out=vT, in_=va[tt_i])
                # [w-in-pair 2, pair, 512] so rhs slices sit at partition 0
                mt = pmask.tile([2, PAIRS, DIM], BF16, tag="mt")
                nc.sync.dma_start(
                    out=mt,
                    in_=ma[tt_i * 2 * PAIRS : (tt_i + 1) * 2 * PAIRS, :].rearrange(
                        "(p w) f -> w p f", w=2
                    ),
                )

                # ---- q projection -> feature-major bf16 [fo 128, c, tok] ----
                yqT = py.tile([128, 4, TT], BF16, tag="yqT")
                for co in range(4):
                    ps = ps_pj.tile([128, TT], F32, tag="pj")
                    for ci in range(4):
                        nc.tensor.matmul(
                            ps,
                            lhsT=w_tiles["wq"][:, ci, ts(co, 128)],
                            rhs=qT[:, ci, :],
                            start=(ci == 0),
                            stop=(ci == 3),
                        )
                    nc.scalar.copy(out=yqT[:, co, :], in_=ps)

                # ---- k projection -> zero-padded per-head-slot layout ----
                # ykTz[0:64, c, 0, :] = head 2c rows, ykTz[64:128, c, 1, :] =
                # head 2c+1 rows, other slots zero -> K=128 S matmuls keep
                # the two heads of a chunk separate.
                ykTz = py.tile([128, 4, 2, TT], BF16, tag="ykTz")
                nc.gpsimd.memset(ykTz[0:64, :, 1, :], 0.0)
                nc.gpsimd.memset(ykTz[64:128, :, 0, :], 0.0)
                for co in range(4):
                    ps = ps_pj.tile([128, TT], F32, tag="pj")
                    for ci in range(4):
                        nc.tensor.matmul(
                            ps,
                            lhsT=w_tiles["wk"][:, ci, ts(co, 128)],
                            rhs=kT[:, ci, :],
                            start=(ci == 0),
                            stop=(ci == 3),
                        )
                    nc.scalar.copy(out=ykTz[0:64, co, 0, :], in_=ps[0:64, :])
                    nc.scalar.copy(out=ykTz[64:128, co, 1, :], in_=ps[64:128, :])

                # ---- v projection -> token-major bf16 [tok%128, pair, fo] ----
                yv = py.tile([128, PAIRS, DIM], BF16, tag="yv")
                for ct in range(PAIRS):
                    ps = ps_pj.tile([128, DIM], F32, tag="pj")
                    for ci in range(4):
                        nc.tensor.matmul(
                            ps,
                            lhsT=vT[:, ci, ts(ct, 128)],
                            rhs=w_tiles["wv"][:, ci, :],
                            start=(ci == 0),
                            stop=(ci == 3),
                        )
                    nc.scalar.copy(out=yv[:, ct, :], in_=ps)
                # second window's vh rows relocated to partition base 0
                yv_hi = py.tile([64, PAIRS, DIM], BF16, tag="yv_hi")
                nc.gpsimd.dma_start(out=yv_hi, in_=yv[64:128, :, :])

                # ---- attention + output projection per 128-token pair ----
                for pr in range(PAIRS):
                    base = pr * 128
                    # S: [pair-tq 128, (head, tk) 512]
                    sp = ps_s.tile([128, HEADS, N], F32, tag="sp")
                    nc.tensor.matmul(
                        sp,
                        lhsT=sel_t,
                        rhs=mt[:, pr, :],
                        start=True,
                        stop=False,
                        skip_group_check=True,
                    )
                    for ch in range(4):
                        for w in range(2):
                            nc.tensor.matmul(
                                sp[ds(64 * w, 64), ds(2 * ch, 2), :],
                                lhsT=yqT[:, ch, ds(base + 64 * w, 64)],
                                rhs=ykTz[:, ch, :, ds(base + 64 * w, 64)],
                                start=False,
                                stop=True,
                                skip_group_check=True,
                            )
                    # softmax (no max-subtraction needed: |S| < 2 or -1e4)
                    pexp = pp.tile([128, HEADS, N], F32, tag="pexp")
                    nc.scalar.activation(
                        out=pexp, in_=sp, func=mybir.ActivationFunctionType.Exp
                    )
                    dsum = pst.tile([128, HEADS], F32, tag="dsum")
                    nc.vector.tensor_reduce(
                        out=dsum,
                        in_=pexp,
                        axis=mybir.AxisListType.X,
                        op=mybir.AluOpType.add,
                    )
                    rec = pst.tile([128, HEADS], F32, tag="rec")
                    nc.vector.reciprocal(out=rec, in_=dsum)
                    # normalized P (dense): pn[(w,tq), h, tk]
                    pn = pp.tile([128, HEADS, N], BF16, tag="pn")
                    rec_b = bass.AP(
                        tensor=rec.tensor,
                        offset=rec.offset,
                        ap=list(rec.ap) + [[0, N]],
                    )
                    nc.vector.tensor_tensor(
                        out=pn, in0=pexp, in1=rec_b, op=mybir.AluOpType.mult
                    )
                    # P.T via PE transpose-mode (slab s: heads 2s,2s+1):
                    # [128 (h-parity, tk), 128 (w, tq)] per slab, then one
                    # ACT copy PSUM->SBUF and a gpsimd relocation so odd
                    # heads' rows also exist at partition base 0.
                    ptp = ps_t.tile([128, 4, 128], BF16, tag="ptp")
                    pn2 = pn.rearrange("p h n -> p (h n)")
                    for s in range(4):
                        nc.tensor.transpose(
                            ptp[:, s, :], pn2[:, ts(s, 128)], id_t
                        )
                    pT = pp.tile([128, 4, 128], BF16, tag="pT")
                    nc.scalar.copy(out=pT, in_=ptp)
                    pT_od = pp.tile([64, 4, 128], BF16, tag="pT_od")
                    nc.gpsimd.dma_start(out=pT_od, in_=pT[64:128, :, :])
                    # X.T: [feat-in-chunk 128, chunk 4, (w, tq) 128]
                    xt = ps_x.tile([128, 4, 128], F32, tag="xt")
                    for h in range(HEADS):
                        ch, hh = h // 2, (h % 2) * 64
                        psrc = pT if h % 2 == 0 else pT_od
                        for w in range(2):
                            vsrc = yv if w == 0 else yv_hi
                            nc.tensor.matmul(
                                xt[ds(hh, 64), ch, ds(64 * w, 64)],
                                lhsT=vsrc[ds(0, 64), pr, ts(h, 64)],
                                rhs=psrc[ds(0, 64), ch, ds(64 * w, 64)],
                                start=True,
                                stop=True,
                            )
                    xts = pp.tile([128, 4, 128], BF16, tag="xts")
                    nc.scalar.copy(out=xts, in_=xt)
                    # output projection: [tok 128, fo 512]
                    po = ps_o.tile([128, DIM], F32, tag="po")
                    for c in range(4):
                        nc.tensor.matmul(
                            po,
                            lhsT=xts[:, c, :],
                            rhs=w_tiles["wp"][:, c, :],
                            start=(c == 0),
                            stop=(c == 3),
                        )
                    osb = pout.tile([128, DIM], F32, tag="osb")
                    nc.scalar.copy(out=osb, in_=po)
                    nc.sync.dma_start(
                        out=oa[t0 + base : t0 + base + 128, :], in_=osb
                    )

    nc.compile()
    return nc


_PROGRAM_CACHE = {}


def _get_program(win_per_core):
    if win_per_core not in _PROGRAM_CACHE:
        _PROGRAM_CACHE[win_per_core] = build_program(win_per_core)
    return _PROGRAM_CACHE[win_per_core]


def _feature_major_tiles(x_flat_bf16):
    """[tok, 512] bf16 -> [n_tt, 128, 4, TT] so each T-tile block is one
    fully-contiguous transposed DMA read."""
    tok = x_flat_bf16.shape[0]
    n_tt = tok // TT
    xt = x_flat_bf16.reshape(n_tt, TT, 4, 128).transpose(0, 3, 2, 1)
    return np.ascontiguousarray(xt)


def make_in_maps(q, k, v, mask, Wq, Wk, Wv, Wp, ncores=NCORES):
    """Host-side shard + layout prep. Returns list of per-core input dicts."""
    B, n, C = q.shape
    win_pc = B // ncores
    qf = np.ascontiguousarray(q.reshape(B * n, C))
    kf = np.ascontiguousarray(k.reshape(B * n, C))
    vf = np.ascontiguousarray(v.reshape(B * n, C))

    # additive mask, repeated per head along the free dim
    madd = np.where(mask == 0, np.float32(MASK_NEG), np.float32(0.0))
    maskrep = np.tile(madd, (1, HEADS)).astype(NP_BF16)  # [B, 512]

    wq_t = np.ascontiguousarray((Wq.T * SCALE).astype(NP_BF16))
    wk_t = np.ascontiguousarray(Wk.T.astype(NP_BF16))
    wv_t = np.ascontiguousarray(Wv.T.astype(NP_BF16))
    wp_t = np.ascontiguousarray(Wp.T.astype(NP_BF16))
    sel = np.zeros((2, 128), NP_BF16)
    sel[0, 0:64] = 1
    sel[1, 64:128] = 1
    ident = np.eye(128, dtype=NP_BF16)

    tok_pc = win_pc * n
    in_maps = []
    for c in range(ncores):
        sl = slice(c * tok_pc, (c + 1) * tok_pc)
        in_maps.append(
            {
                "qT": _feature_major_tiles(qf[sl].astype(NP_BF16)),
                "kT": _feature_major_tiles(kf[sl].astype(NP_BF16)),
                "vT": _feature_major_tiles(vf[sl].astype(NP_BF16)),
                "maskrep": maskrep[c * win_pc : (c + 1) * win_pc],
                "wq": wq_t,
                "wk": wk_t,
                "wv": wv_t,
                "wp": wp_t,
                "sel01": sel,
                "ident": ident,
            }
        )
    return in_maps


def _reference_numpy(q, k, v, mask, Wq, bq, Wk, bk, Wv, bv, Wp, bp):
    """Full-precision host fallback (only used for nonzero bq/bk)."""
    B, n, C = q.shape
    qh = (q.reshape(-1, C) @ Wq.T + bq).reshape(B, n, HEADS, HD).transpose(0, 2, 1, 3)
    kh = (k.reshape(-1, C) @ Wk.T + bk).reshape(B, n, HEADS, HD).transpose(0, 2, 1, 3)
    vh = (v.reshape(-1, C) @ Wv.T + bv).reshape(B, n, HEADS, HD).transpose(0, 2, 1, 3)
    s = np.einsum("bhqd,bhkd->bhqk", qh * SCALE, kh)
    s = np.where((mask[:, None, None, :] == 0), np.float32(MASK_NEG), s)
    s = s - s.max(-1, keepdims=True)
    e = np.exp(s)
    p = e / e.sum(-1, keepdims=True)
    x = np.einsum("bhqk,bhkd->bhqd", p, vh)
    x = x.transpose(0, 2, 1, 3).reshape(B, n, C)
    return (x @ Wp.T + bp).astype(np.float32)


def kernel(q, k, v, mask, Wq, bq, Wk, bk, Wv, bv, Wp, bp, trace=False):
    q = np.asarray(q, np.float32)
    k = np.asarray(k, np.float32)
    v = np.asarray(v, np.float32)
    mask = np.asarray(mask)
    Wq, Wk, Wv, Wp = (np.asarray(w, np.float32) for w in (Wq, Wk, Wv, Wp))
    bq, bk, bv, bp = (np.asarray(b, np.float32) for b in (bq, bk, bv, bp))

    if np.any(bq) or np.any(bk):
        return _reference_numpy(q, k, v, mask, Wq, bq, Wk, bk, Wv, bv, Wp, bp)

    B, n, C = q.shape
    win_pc = B // NCORES
    nc = _get_program(win_pc)
    in_maps = make_in_maps(q, k, v, mask, Wq, Wk, Wv, Wp)
    res = run_bass_kernel_spmd(
        nc, in_maps, core_ids=list(range(NCORES)), trace=trace
    )
    outs = np.concatenate([r["out"] for r in res.results], axis=0)
    outs = outs.reshape(B, n, C)
    # bv flows through attention linearly (softmax rows sum to 1); with bp it
    # folds into a single output bias.
    bout = bp + bv @ Wp.T
    if np.any(bout):
        outs = outs + bout.astype(np.float32)
    if trace:
        kernel._last_result = res
    return outs


# revision 18
# speedup vs baseline: 2.8894x; 1.0853x over previous
"""Bass/Trainium2 kernel for windowed multi-head attention (8 NeuronCores).

Math per window b (64 tokens), matching torch-style nn.MultiHeadAttention:
  qh = (q @ Wq.T + bq) * scale; kh = k @ Wk.T + bk; vh = v @ Wv.T + bv
  S  = qh @ kh.T  (per head);  S[masked k] = -1e4;  P = softmax(S)
  out = concat_h(P @ vh) @ Wp.T + bp

Device dataflow (per core, 256 windows = 16384 tokens, all matmuls bf16
with f32 PSUM accumulation):
  - q,k,v are pre-transposed on the host to feature-major [128, 4, TT]
    T-tile blocks so input DMAs are single fully-contiguous 512KB reads.
  - Projections on PE: q -> feature-major Y.T chunks; k -> feature-major
    zero-padded per-head-slot layout; v -> token-major.
  - Per window-pair (128 tokens): S for all 8 heads via 8 K=128 matmuls
    (the zero-padded k layout keeps the two heads of a chunk separate),
    additive mask applied as one rank-2 matmul (sel01 x maskrep),
    exp on ACT, per-head row-sum + reciprocal + normalize on DVE writing
    a zero-padded P layout, P.T via 8 DMA-xbar transposes (split across
    both HWDGE queues), X.T via 8 K=128 matmuls against block-diagonal
    P.T, then the output projection directly from X.T chunks.

All matmul operands sit at partition base 0 with K in {2, 128}: mixing
row-group offsets across back-to-back K<128 matmuls crashes the PE.

Biases are zero in this problem; nonzero bv/bp fold into a host-side
output add, nonzero bq/bk fall back to a host computation.
"""

import sys

for _p in ("/opt/trn_rl_repo",):
    if _p not in sys.path:
        sys.path.append(_p)

import numpy as np
import ml_dtypes

import concourse.bacc as bacc
import concourse.bass as bass
import concourse.mybir as mybir
import concourse.tile as tile
from concourse.bass import ds, ts
from concourse.bass_utils import run_bass_kernel_spmd

BF16 = mybir.dt.bfloat16
F32 = mybir.dt.float32
NP_BF16 = ml_dtypes.bfloat16

NCORES = 8
DIM = 512
HEADS = 8
HD = 64
N = 64  # window length
B_TOTAL = 2048
WIN_PER_CORE = B_TOTAL // NCORES  # 256
TOK_PER_CORE = WIN_PER_CORE * N  # 16384
SCALE = HD ** -0.5
MASK_NEG = -10000.0
TT = 512  # token tile (8 windows, 4 pairs)


def build_program(win_per_core=WIN_PER_CORE):
    """Build the per-core Tile program. All 8 cores run it SPMD."""
    tok = win_per_core * N
    n_tt = tok // TT
    PAIRS = TT // 128  # 4

    nc = bacc.Bacc("TRN2", debug=False)

    qT_d = nc.dram_tensor("qT", [n_tt, 128, 4, TT], BF16, kind="ExternalInput")
    kT_d = nc.dram_tensor("kT", [n_tt, 128, 4, TT], BF16, kind="ExternalInput")
    vT_d = nc.dram_tensor("vT", [n_tt, 128, 4, TT], BF16, kind="ExternalInput")
    maskrep = nc.dram_tensor(
        "maskrep", [win_per_core, DIM], BF16, kind="ExternalInput"
    )
    wq = nc.dram_tensor("wq", [DIM, DIM], BF16, kind="ExternalInput")
    wk = nc.dram_tensor("wk", [DIM, DIM], BF16, kind="ExternalInput")
    wv = nc.dram_tensor("wv", [DIM, DIM], BF16, kind="ExternalInput")
    wp = nc.dram_tensor("wp", [DIM, DIM], BF16, kind="ExternalInput")
    sel01 = nc.dram_tensor("sel01", [2, 128], BF16, kind="ExternalInput")
    ident = nc.dram_tensor("ident", [128, 128], BF16, kind="ExternalInput")
    out = nc.dram_tensor("out", [tok, DIM], F32, kind="ExternalOutput")

    qa, ka, va, ma = qT_d.ap(), kT_d.ap(), vT_d.ap(), maskrep.ap()
    oa = out.ap()

    with tile.TileContext(nc) as tc:
        with (
            tc.tile_pool(name="consts", bufs=1) as consts,
            tc.tile_pool(name="pin", bufs=2) as pin,
            tc.tile_pool(name="py", bufs=2) as py,
            tc.tile_pool(name="pmask", bufs=2) as pmask,
            tc.tile_pool(name="pp", bufs=3) as pp,
            tc.tile_pool(name="pst", bufs=4) as pst,
            tc.tile_pool(name="pout", bufs=3) as pout,
            tc.tile_pool(name="ps_pj", bufs=2, space="PSUM") as ps_pj,
            tc.tile_pool(name="ps_s", bufs=2, space="PSUM") as ps_s,
            tc.tile_pool(name="ps_t", bufs=1, space="PSUM") as ps_t,
            tc.tile_pool(name="ps_x", bufs=2, space="PSUM") as ps_x,
            tc.tile_pool(name="ps_o", bufs=1, space="PSUM") as ps_o,
        ):
            # Weights: [fi_part 128, fi_chunk 4, fo 512]
            w_tiles = {}
            for name, t in (("wq", wq), ("wk", wk), ("wv", wv), ("wp", wp)):
                wt = consts.tile([128, 4, DIM], BF16, tag=f"w_{name}")
                nc.sync.dma_start(
                    out=wt, in_=t.ap().rearrange("(c p) f -> p c f", p=128)
                )
                w_tiles[name] = wt
            sel_t = consts.tile([2, 128], BF16, tag="sel01")
            nc.sync.dma_start(out=sel_t, in_=sel01.ap())
            id_t = consts.tile([128, 128], BF16, tag="ident")
            nc.sync.dma_start(out=id_t, in_=ident.ap())

            for tt_i in range(n_tt):
                t0 = tt_i * TT
                # ---- transposed input loads (host pre-transposed) ----
                qT = pin.tile([128, 4, TT], BF16, tag="qT")
                kT = pin.tile([128, 4, TT], BF16, tag="kT")
                vT = pin.tile([128, 4, TT], BF16, tag="vT")
                nc.sync.dma_start(out=qT, in_=qa[tt_i])
                nc.sync.dma_start(out=kT, in_=ka[tt_i])
                nc.sync.dma_start(out=vT, in_=va[tt_i])
                # [w-in-pair 2, pair, 512] so rhs slices sit at partition 0
                mt = pmask.tile([2, PAIRS, DIM], BF16, tag="mt")
                nc.sync.dma_start(
                    out=mt,
                    in_=ma[tt_i * 2 * PAIRS : (tt_i + 1) * 2 * PAIRS, :].rearrange(
                        "(p w) f -> w p f", w=2
                    ),
                )

                # ---- q projection -> feature-major bf16 [fo 128, c, tok] ----
                yqT = py.tile([128, 4, TT], BF16, tag="yqT")
                for co in range(4):
                    ps = ps_pj.tile([128, TT], F32, tag="pj")
                    for ci in range(4):
                        nc.tensor.matmul(
                            ps,
                            lhsT=w_tiles["wq"][:, ci, ts(co, 128)],
                            rhs=qT[:, ci, :],
                            start=(ci == 0),
                            stop=(ci == 3),
                        )
                    nc.scalar.copy(out=yqT[:, co, :], in_=ps)

                # ---- k projection -> zero-padded per-head-slot layout ----
                # ykTz[0:64, c, 0, :] = head 2c rows, ykTz[64:128, c, 1, :] =
                # head 2c+1 rows, other slots zero -> K=128 S matmuls keep
                # the two heads of a chunk separate.
                ykTz = py.tile([128, 4, 2, TT], BF16, tag="ykTz")
                nc.gpsimd.memset(ykTz[0:64, :, 1, :], 0.0)
                nc.gpsimd.memset(ykTz[64:128, :, 0, :], 0.0)
                for co in range(4):
                    ps = ps_pj.tile([128, TT], F32, tag="pj")
                    for ci in range(4):
                        nc.tensor.matmul(
                            ps,
                            lhsT=w_tiles["wk"][:, ci, ts(co, 128)],
                            rhs=kT[:, ci, :],
                            start=(ci == 0),
                            stop=(ci == 3),
                        )
                    nc.scalar.copy(out=ykTz[0:64, co, 0, :], in_=ps[0:64, :])
                    nc.scalar.copy(out=ykTz[64:128, co, 1, :], in_=ps[64:128, :])

                # ---- v projection -> token-major bf16 [tok%128, pair, fo] ----
                yv = py.tile([128, PAIRS, DIM], BF16, tag="yv")
                for ct in range(PAIRS):
                    ps = ps_pj.tile([128, DIM], F32, tag="pj")
                    for ci in range(4):
                        nc.tensor.matmul(
                            ps,
                            lhsT=vT[:, ci, ts(ct, 128)],
                            rhs=w_tiles["wv"][:, ci, :],
                            start=(ci == 0),
                            stop=(ci == 3),
                        )
                    nc.scalar.copy(out=yv[:, ct, :], in_=ps)
                # second window's vh rows relocated to partition base 0
                yv_hi = py.tile([64, PAIRS, DIM], BF16, tag="yv_hi")
                nc.gpsimd.dma_start(out=yv_hi, in_=yv[64:128, :, :])

                # ---- attention + output projection per 128-token pair ----
                for pr in range(PAIRS):
                    base = pr * 128
                    # S: [pair-tq 128, (head, tk) 512]
                    sp = ps_s.tile([128, HEADS, N], F32, tag="sp")
                    nc.tensor.matmul(
                        sp,
                        lhsT=sel_t,
                        rhs=mt[:, pr, :],
                        start=True,
                        stop=False,
                        skip_group_check=True,
                    )
                    for ch in range(4):
                        for w in range(2):
                            nc.tensor.matmul(
                                sp[ds(64 * w, 64), ds(2 * ch, 2), :],
                                lhsT=yqT[:, ch, ds(base + 64 * w, 64)],
                                rhs=ykTz[:, ch, :, ds(base + 64 * w, 64)],
                                start=False,
                                stop=True,
                                skip_group_check=True,
                            )
                    # softmax (no max-subtraction needed: |S| < 2 or -1e4)
                    pexp = pp.tile([128, HEADS, N], BF16, tag="pexp")
                    nc.scalar.activation(
                        out=pexp, in_=sp, func=mybir.ActivationFunctionType.Exp
                    )
                    dsum = pst.tile([128, HEADS], F32, tag="dsum")
                    nc.vector.tensor_reduce(
                        out=dsum,
                        in_=pexp,
                        axis=mybir.AxisListType.X,
                        op=mybir.AluOpType.add,
                    )
                    rec = pst.tile([128, HEADS], F32, tag="rec")
                    nc.vector.reciprocal(out=rec, in_=dsum)
                    # normalized P (dense): pn[(w,tq), h, tk]
                    pn = pp.tile([128, HEADS, N], BF16, tag="pn")
                    rec_b = bass.AP(
                        tensor=rec.tensor,
                        offset=rec.offset,
                        ap=list(rec.ap) + [[0, N]],
                    )
                    nc.vector.tensor_tensor(
                        out=pn, in0=pexp, in1=rec_b, op=mybir.AluOpType.mult
                    )
                    # P.T via PE transpose-mode (slab s: heads 2s,2s+1):
                    # [128 (h-parity, tk), 128 (w, tq)] per slab, then one
                    # ACT copy PSUM->SBUF and a gpsimd relocation so odd
                    # heads' rows also exist at partition base 0.
                    ptp = ps_t.tile([128, 4, 128], BF16, tag="ptp")
                    pn2 = pn.rearrange("p h n -> p (h n)")
                    for s in range(4):
                        nc.tensor.transpose(
                            ptp[:, s, :], pn2[:, ts(s, 128)], id_t
                        )
                    pT = pp.tile([128, 4, 128], BF16, tag="pT")
                    nc.scalar.copy(out=pT, in_=ptp)
                    pT_od = pp.tile([64, 4, 128], BF16, tag="pT_od")
                    nc.gpsimd.dma_start(out=pT_od, in_=pT[64:128, :, :])
                    # X.T: [feat-in-chunk 128, chunk 4, (w, tq) 128]
                    xt = ps_x.tile([128, 4, 128], F32, tag="xt")
                    for h in range(HEADS):
                        ch, hh = h // 2, (h % 2) * 64
                        psrc = pT if h % 2 == 0 else pT_od
                        for w in range(2):
                            vsrc = yv if w == 0 else yv_hi
                            nc.tensor.matmul(
                                xt[ds(hh, 64), ch, ds(64 * w, 64)],
                                lhsT=vsrc[ds(0, 64), pr, ts(h, 64)],
                                rhs=psrc[ds(0, 64), ch, ds(64 * w, 64)],
                                start=True,
                                stop=True,
                            )
                    xts = pp.tile([128, 4, 128], BF16, tag="xts")
                    nc.vector.tensor_copy(out=xts, in_=xt)
                    # output projection: [tok 128, fo 512]
                    po = ps_o.tile([128, DIM], F32, tag="po")
                    for c in range(4):
                        nc.tensor.matmul(
                            po,
                            lhsT=xts[:, c, :],
                            rhs=w_tiles["wp"][:, c, :],
                            start=(c == 0),
                            stop=(c == 3),
                        )
                    osb = pout.tile([128, DIM], F32, tag="osb")
                    nc.vector.tensor_copy(out=osb, in_=po)
                    nc.sync.dma_start(
                        out=oa[t0 + base : t0 + base + 128, :], in_=osb
                    )

    nc.compile()
    return nc


_PROGRAM_CACHE = {}


def _get_program(win_per_core):
    if win_per_core not in _PROGRAM_CACHE:
        _PROGRAM_CACHE[win_per_core] = build_program(win_per_core)
    return _PROGRAM_CACHE[win_per_core]


def _feature_major_tiles(x_flat_bf16):
    """[tok, 512] bf16 -> [n_tt, 128, 4, TT] so each T-tile block is one
    fully-contiguous transposed DMA read."""
    tok = x_flat_bf16.shape[0]
    n_tt = tok // TT
    xt = x_flat_bf16.reshape(n_tt, TT, 4, 128).transpose(0, 3, 2, 1)
    return np.ascontiguousarray(xt)


def make_in_maps(q, k, v, mask, Wq, Wk, Wv, Wp, ncores=NCORES):
    """Host-side shard + layout prep. Returns list of per-core input dicts."""
    B, n, C = q.shape
    win_pc = B // ncores
    qf = np.ascontiguousarray(q.reshape(B * n, C))
    kf = np.ascontiguousarray(k.reshape(B * n, C))
    vf = np.ascontiguousarray(v.reshape(B * n, C))

    # additive mask, repeated per head along the free dim
    madd = np.where(mask == 0, np.float32(MASK_NEG), np.float32(0.0))
    maskrep = np.tile(madd, (1, HEADS)).astype(NP_BF16)  # [B, 512]

    wq_t = np.ascontiguousarray((Wq.T * SCALE).astype(NP_BF16))
    wk_t = np.ascontiguousarray(Wk.T.astype(NP_BF16))
    wv_t = np.ascontiguousarray(Wv.T.astype(NP_BF16))
    wp_t = np.ascontiguousarray(Wp.T.astype(NP_BF16))
    sel = np.zeros((2, 128), NP_BF16)
    sel[0, 0:64] = 1
    sel[1, 64:128] = 1
    ident = np.eye(128, dtype=NP_BF16)

    tok_pc = win_pc * n
    in_maps = []
    for c in range(ncores):
        sl = slice(c * tok_pc, (c + 1) * tok_pc)
        in_maps.append(
            {
                "qT": _feature_major_tiles(qf[sl].astype(NP_BF16)),
                "kT": _feature_major_tiles(kf[sl].astype(NP_BF16)),
                "vT": _feature_major_tiles(vf[sl].astype(NP_BF16)),
                "maskrep": maskrep[c * win_pc : (c + 1) * win_pc],
                "wq": wq_t,
                "wk": wk_t,
                "wv": wv_t,
                "wp": wp_t,
                "sel01": sel,
                "ident": ident,
            }
        )
    return in_maps


def _reference_numpy(q, k, v, mask, Wq, bq, Wk, bk, Wv, bv, Wp, bp):
    """Full-precision host fallback (only used for nonzero bq/bk)."""
    B, n, C = q.shape
    qh = (q.reshape(-1, C) @ Wq.T + bq).reshape(B, n, HEADS, HD).transpose(0, 2, 1, 3)
    kh = (k.reshape(-1, C) @ Wk.T + bk).reshape(B, n, HEADS, HD).transpose(0, 2, 1, 3)
    vh = (v.reshape(-1, C) @ Wv.T + bv).reshape(B, n, HEADS, HD).transpose(0, 2, 1, 3)
    s = np.einsum("bhqd,bhkd->bhqk", qh * SCALE, kh)
    s = np.where((mask[:, None, None, :] == 0), np.float32(MASK_NEG), s)
    s = s - s.max(-1, keepdims=True)
    e = np.exp(s)
    p = e / e.sum(-1, keepdims=True)
    x = np.einsum("bhqk,bhkd->bhqd", p, vh)
    x = x.transpose(0, 2, 1, 3).reshape(B, n, C)
    return (x @ Wp.T + bp).astype(np.float32)


def kernel(q, k, v, mask, Wq, bq, Wk, bk, Wv, bv, Wp, bp, trace=False):
    q = np.asarray(q, np.float32)
    k = np.asarray(k, np.float32)
    v = np.asarray(v, np.float32)
    mask = np.asarray(mask)
    Wq, Wk, Wv, Wp = (np.asarray(w, np.float32) for w in (Wq, Wk, Wv, Wp))
    bq, bk, bv, bp = (np.asarray(b, np.float32) for b in (bq, bk, bv, bp))

    if np.any(bq) or np.any(bk):
        return _reference_numpy(q, k, v, mask, Wq, bq, Wk, bk, Wv, bv, Wp, bp)

    B, n, C = q.shape
    win_pc = B // NCORES
    nc = _get_program(win_pc)
    in_maps = make_in_maps(q, k, v, mask, Wq, Wk, Wv, Wp)
    res = run_bass_kernel_spmd(
        nc, in_maps, core_ids=list(range(NCORES)), trace=trace
    )
    outs = np.concatenate([r["out"] for r in res.results], axis=0)
    outs = outs.reshape(B, n, C)
    # bv flows through attention linearly (softmax rows sum to 1); with bp it
    # folds into a single output bias.
    bout = bp + bv @ Wp.T
    if np.any(bout):
        outs = outs + bout.astype(np.float32)
    if trace:
        kernel._last_result = res
    return outs
